# revision 25
# baseline (speedup 1.0000x reference)
"""Binarized complex-style dense layer on 8 TRN2 NeuronCores.

Computes out = sign(x + eps) @ K^T with K = [[br, -bi], [bi, br]],
br = sign(weight_real + eps), bi = sign(weight_imag + eps).

Sharding: data-parallel over the batch dim (131072 rows -> 16384 per core),
weights replicated.  Forward only, so no collectives.

Each core receives its x shard as [128, 2*16384] bf16: row k' in [0,128),
column 2b + t holding x[batch row b, 128t + k'] (+eps folded on the host --
sign-safe since bf16 round-to-nearest never crosses zero).  That pair
interleave is exactly the fp8 DoubleRow moving-operand format, so the PE
does the full k=256 contraction in ONE matmul per 512 output columns at
2 MACs/cell/cycle:

  DMA in   : per 2048-col chunk one 1 MB load, 8 KB contiguous/partition
  binarize : one DVE tensor_scalar per chunk: (x >= 0) - 0.5 -> {-.5,+.5} fp8
             (kernel weights scaled to {-2,+2} fp8 so products are exactly +-1)
  PE       : DoubleRow matmuls, rhs N=1024 (512 b-pairs), stationary =
             [128, 2, 128] fp8 piece of kernelT per o-half
  PSUM     : outT [o, b] f32, exact even ints in [-256, 256]
  copy     : ACT/DVE f32 -> int8 with scale 0.5 over [128, 1024] 2-bank APs
  DMA out  : transposed int8 out/2; host untransposes and upcasts *2
"""

import sys

import numpy as np

try:
    import concourse.bass  # noqa: F401
except ImportError:  # fresh env without the axon PYTHONPATH entries
    for p in ("/root/.axon_site/_ro/trn_rl_repo", "/opt/trn_rl_repo"):
        if p not in sys.path:
            sys.path.append(p)

import ml_dtypes

N_CORES = 8
B_TOTAL = 131072
ROWS_PER_CORE = B_TOTAL // N_CORES  # 16384
FAN = 128
K2 = 2 * FAN  # 256 = 2*fan_in = 2*fan_out
EPS = 1e-6
CHUNKS = [512, 512, 1024] + [2048] * 6 + [1024, 512, 512]
assert sum(CHUNKS) == ROWS_PER_CORE

_NC_CACHE = {}


def _build_nc(rows_per_core):
    from concourse import bacc, mybir, tile

    f32 = mybir.dt.float32
    bf16 = mybir.dt.bfloat16
    fp8 = mybir.dt.float8e4
    i8 = mybir.dt.int8
    Copy = mybir.ActivationFunctionType.Copy
    Alu = mybir.AluOpType
    DR = mybir.MatmulPerfMode.DoubleRow

    assert rows_per_core == ROWS_PER_CORE
    starts = [sum(CHUNKS[:i]) for i in range(len(CHUNKS))]

    nc = bacc.Bacc("TRN2", target_bir_lowering=False, debug=False)

    # x arrives k-major: row k (0..255), column b = batch row within shard.
    x_d = nc.dram_tensor("x", [K2, rows_per_core], bf16, kind="ExternalInput")
    # weights arrive pre-transposed: wrt[k, o] = weight_real[o, k].
    wrt_d = nc.dram_tensor("weight_real_t", [FAN, FAN], f32, kind="ExternalInput")
    wit_d = nc.dram_tensor("weight_imag_t", [FAN, FAN], f32, kind="ExternalInput")
    # out is produced transposed: out_d[o, b] = out[b, o] / 2.
    out_d = nc.dram_tensor("out", [K2, rows_per_core], i8, kind="ExternalOutput")

    with tile.TileContext(nc) as tc:
        with (
            tc.tile_pool(name="const", bufs=1) as const_pool,
            tc.tile_pool(name="xt", bufs=len(CHUNKS)) as xt_pool,
            tc.tile_pool(name="xb", bufs=4) as xb_pool,
            tc.tile_pool(name="oout", bufs=5) as o_pool,
            tc.tile_pool(name="po", bufs=4, space="PSUM") as po_pool,
        ):
            # Per-chunk input tiles [128, (t, w)]: k-half t on cols t*w + b.
            xt_tiles = []

            def load(c):
                s, w = starts[c], CHUNKS[c]
                xt = xt_pool.tile([128, 2 * w], bf16, tag="xt")
                nc.sync.dma_start(
                    out=xt[:].rearrange("p (t b) -> p t b", t=2),
                    in_=x_d.rearrange("(t p) b -> p t b", t=2)[:, :, s : s + w],
                )
                xt_tiles.append(xt)

            # Get the x stream going before anything else.
            for c in range(len(CHUNKS)):
                load(c)

            # PE warm-up: ~6us of junk matmuls so the HAM clock gate opens
            # (1.2 -> 2.4 GHz) before the first real matmul arrives, with no
            # >3.4us PE-idle window in between.
            warm = const_pool.tile([128, 128], bf16)
            nc.gpsimd.memset(warm[:], 1.0)
            warm_ps = po_pool.tile([128, 1024], f32, tag="po", name="warm_ps")
            for _ in range(90):
                nc.tensor.matmul(
                    warm_ps[:, 0:64], warm[:], warm[:, 0:64], start=True, stop=True
                )

            # DoubleRow stationary tiles, one per o-half s, laid out
            # [k', (t, oo)] with t = k-half:
            #   ktd0 = [ 2*sign(wrT) | -2*sign(wiT) ]   (o in [0,128))
            #   ktd1 = [ 2*sign(wiT) |  2*sign(wrT) ]   (o in [128,256))
            # sign(w + eps) = (w >= -eps)*2 - 1, two tensor_scalar passes on
            # GPSIMD (keeps the DVE free for the first binarize).
            w_sb = const_pool.tile([128, 256], f32)
            nc.scalar.dma_start(out=w_sb[:, 0:128], in_=wrt_d[:])
            nc.scalar.dma_start(out=w_sb[:, 128:256], in_=wit_d[:])
            kt_t = const_pool.tile([128, 512], bf16)
            # order: [wrT+, wiT-, wiT+, wrT+] scaled {0,+-4}
            nc.vector.tensor_scalar(kt_t[:, 0:128], w_sb[:, 0:128], -EPS, 4.0, Alu.is_ge, Alu.mult)
            nc.vector.tensor_scalar(kt_t[:, 128:256], w_sb[:, 128:256], -EPS, -4.0, Alu.is_ge, Alu.mult)
            nc.vector.tensor_scalar(kt_t[:, 256:384], w_sb[:, 128:256], -EPS, 4.0, Alu.is_ge, Alu.mult)
            nc.vector.tensor_scalar(kt_t[:, 384:512], w_sb[:, 0:128], -EPS, 4.0, Alu.is_ge, Alu.mult)
            # Stationary layout [p, (s, t, o)]: for each o-half s the
            # weights AP is [Ki, (t: n=2, step 128), (o: step 1)] -- the
            # DoubleRow format (first free dim = the 2 k-halves).
            ktd = const_pool.tile([128, 512], fp8)
            nc.vector.tensor_scalar(ktd[:, 0:128], kt_t[:, 0:128], -2.0, None, Alu.add)
            nc.vector.tensor_scalar(ktd[:, 128:256], kt_t[:, 128:256], 2.0, None, Alu.add)
            nc.vector.tensor_scalar(ktd[:, 256:512], kt_t[:, 256:512], -2.0, None, Alu.add)
            ktdv = ktd[:].rearrange("p (s t o) -> p s t o", s=2, t=2)

            pending_stores = []
            for c in range(len(CHUNKS)):
                s, w = starts[c], CHUNKS[c]
                xbt = xb_pool.tile([128, 2 * w], fp8, tag="xb")
                xbv = xbt[:].rearrange("p (t b) -> p t b", t=2)
                # One-shot binarize: (x >= 0) - 0.5 -> {-0.5, +0.5} fp8.
                nc.vector.tensor_scalar(
                    xbt[:], xt_tiles[c][:], 0.0, 0.5, Alu.is_ge, Alu.subtract
                )
                ot = o_pool.tile([128, 2 * w], i8, tag="ot")
                otv = ot[:].rearrange("p (os b) -> p os b", os=2)
                # po tiles of up to 1024 cols (2 PSUM banks)
                widths = []
                b = 0
                while b < w:
                    wd = min(1024, w - b)
                    widths.append((b, wd))
                    b += wd
                for os_half in range(2):
                    pos = [
                        po_pool.tile([128, wd], f32, tag="po", name=f"po_{c}_{os_half}_{j}")
                        for j, (_, wd) in enumerate(widths)
                    ]
                    for j, (jb, wd) in enumerate(widths):
                        for q in range(wd // 512):
                            # DoubleRow: rhs = [p, (t: 2, step w), (b: 512)],
                            # stationary holds both k-halves; one matmul =
                            # full k=256 contraction for 512 batch rows.
                            b0 = jb + q * 512
                            nc.tensor.matmul(
                                pos[j][:, q * 512 : (q + 1) * 512],
                                ktdv[:, os_half, :, :],
                                xbv[:, :, b0 : b0 + 512],
                                start=True,
                                stop=True,
                                perf_mode=DR,
                            )
                    for j, (jb, wd) in enumerate(widths):
                        dst = otv[:, os_half, jb : jb + wd]
                        # ~7 of 32 copies on DVE (emitted before the next
                        # chunk's binarize in the FIFO), rest on ACT.
                        on_dve = (os_half == 0 and j == 0 and w == 2048) or (
                            os_half == 0 and j == 1 and c == 6
                        )
                        if on_dve:
                            nc.vector.tensor_scalar(dst, pos[j][:], 0.5, None, Alu.mult)
                        else:
                            nc.scalar.activation(dst, pos[j][:], Copy, bias=0.0, scale=0.5)
                # Defer the store by one chunk: its issue then can't
                # steal SDMA bandwidth from the next chunk's load during
                # the ramp (stores have plenty of slack).
                def _store(s=s, w=w, otv=otv):
                    nc.gpsimd.dma_start(
                        out=out_d.rearrange("(os p) b -> p os b", os=2)[:, :, s : s + w],
                        in_=otv,
                    )
                pending_stores.append(_store)
                if len(pending_stores) > 1:
                    pending_stores.pop(0)()
            for ps in pending_stores:
                ps()

    nc.compile()
    return nc


def get_nc(rows_per_core=ROWS_PER_CORE):
    if rows_per_core not in _NC_CACHE:
        _NC_CACHE[rows_per_core] = _build_nc(rows_per_core)
    return _NC_CACHE[rows_per_core]


def kernel(x, weight_real, weight_imag, trace=False, tmpdir=None):
    from concourse import bass_utils

    x = np.asarray(x, dtype=np.float32)
    wr = np.asarray(weight_real, dtype=np.float32)
    wi = np.asarray(weight_imag, dtype=np.float32)
    assert x.shape == (B_TOTAL, K2) and wr.shape == (FAN, FAN) and wi.shape == (FAN, FAN)

    # Fold the +eps into the bf16 cast: sign(bf16(x + eps)) == sign(x + eps)
    # (round-to-nearest never crosses 0; exact-0 results go +1 via the
    # device-side >= 0 test, matching sign(0 + eps)).  Feed each core its
    # shard k-major ([256, 16384]); weights go in pre-transposed.
    x_bf = (x + np.float32(EPS)).astype(ml_dtypes.bfloat16)
    xp = np.ascontiguousarray(
        x_bf.reshape(N_CORES, ROWS_PER_CORE, K2).transpose(0, 2, 1)
    )
    wrt = np.ascontiguousarray(wr.T)
    wit = np.ascontiguousarray(wi.T)

    nc = get_nc()
    in_maps = [
        {"x": xp[i], "weight_real_t": wrt, "weight_imag_t": wit}
        for i in range(N_CORES)
    ]
    res = bass_utils.run_bass_kernel_spmd(
        nc, in_maps, core_ids=list(range(N_CORES)), trace=trace, tmpdir=tmpdir
    )
    # out_d[o, b] = out[b, o]/2 per core: untranspose and upcast.
    out = np.empty((B_TOTAL, K2), dtype=np.float32)
    for i in range(N_CORES):
        np.multiply(
            res.results[i]["out"].T, np.float32(2.0),
            out=out[i * ROWS_PER_CORE : (i + 1) * ROWS_PER_CORE],
        )
    if trace:
        return out, res
    return out


# revision 26
# speedup vs baseline: 1.1493x; 1.1493x over previous
"""Binarized complex-style dense layer on 8 TRN2 NeuronCores.

Computes out = sign(x + eps) @ K^T with K = [[br, -bi], [bi, br]],
br = sign(weight_real + eps), bi = sign(weight_imag + eps).

Sharding: data-parallel over the batch dim (131072 rows -> 16384 per core),
weights replicated.  Forward only, so no collectives.

Each core receives its x shard as [128, 2*16384] bf16: row k' in [0,128),
column 2b + t holding x[batch row b, 128t + k'] (+eps folded on the host --
sign-safe since bf16 round-to-nearest never crosses zero).  That pair
interleave is exactly the fp8 DoubleRow moving-operand format, so the PE
does the full k=256 contraction in ONE matmul per 512 output columns at
2 MACs/cell/cycle:

  DMA in   : per 2048-col chunk one 1 MB load, 8 KB contiguous/partition
  binarize : one DVE tensor_scalar per chunk: (x >= 0) - 0.5 -> {-.5,+.5} fp8
             (kernel weights scaled to {-2,+2} fp8 so products are exactly +-1)
  PE       : DoubleRow matmuls, rhs N=1024 (512 b-pairs), stationary =
             [128, 2, 128] fp8 piece of kernelT per o-half
  PSUM     : outT [o, b] f32, exact even ints in [-256, 256]
  copy     : ACT/DVE f32 -> int8 with scale 0.5 over [128, 1024] 2-bank APs
  DMA out  : transposed int8 out/2; host untransposes and upcasts *2
"""

import sys

import numpy as np

try:
    import concourse.bass  # noqa: F401
except ImportError:  # fresh env without the axon PYTHONPATH entries
    for p in ("/root/.axon_site/_ro/trn_rl_repo", "/opt/trn_rl_repo"):
        if p not in sys.path:
            sys.path.append(p)

import ml_dtypes

N_CORES = 8
B_TOTAL = 131072
ROWS_PER_CORE = B_TOTAL // N_CORES  # 16384
FAN = 128
K2 = 2 * FAN  # 256 = 2*fan_in = 2*fan_out
EPS = 1e-6
CHUNKS = [512, 512, 1024] + [2048] * 6 + [1024, 512, 512]
assert sum(CHUNKS) == ROWS_PER_CORE

_NC_CACHE = {}


def _build_nc(rows_per_core):
    from concourse import bacc, mybir, tile

    f32 = mybir.dt.float32
    bf16 = mybir.dt.bfloat16
    fp8 = mybir.dt.float8e4
    i8 = mybir.dt.int8
    Copy = mybir.ActivationFunctionType.Copy
    Alu = mybir.AluOpType
    DR = mybir.MatmulPerfMode.DoubleRow

    assert rows_per_core == ROWS_PER_CORE
    starts = [sum(CHUNKS[:i]) for i in range(len(CHUNKS))]

    nc = bacc.Bacc("TRN2", target_bir_lowering=False, debug=False)

    # x arrives k-major: row k (0..255), column b = batch row within shard.
    x_d = nc.dram_tensor("x", [K2, rows_per_core], bf16, kind="ExternalInput")
    # weights arrive pre-transposed: wrt[k, o] = weight_real[o, k].
    wrt_d = nc.dram_tensor("weight_real_t", [FAN, FAN], f32, kind="ExternalInput")
    wit_d = nc.dram_tensor("weight_imag_t", [FAN, FAN], f32, kind="ExternalInput")
    # out is produced transposed: out_d[o, b] = out[b, o] / 2.
    out_d = nc.dram_tensor("out", [K2, rows_per_core], i8, kind="ExternalOutput")

    with tile.TileContext(nc) as tc:
        with (
            tc.tile_pool(name="const", bufs=1) as const_pool,
            tc.tile_pool(name="xt", bufs=len(CHUNKS)) as xt_pool,
            tc.tile_pool(name="xb", bufs=4) as xb_pool,
            tc.tile_pool(name="oout", bufs=4) as o_pool,
            tc.tile_pool(name="po", bufs=4, space="PSUM") as po_pool,
        ):
            # Per-chunk input tiles [128, (t, w)]: k-half t on cols t*w + b.
            xt_tiles = []

            def load(c):
                s, w = starts[c], CHUNKS[c]
                xt = xt_pool.tile([128, 2 * w], bf16, tag="xt")
                nc.sync.dma_start(
                    out=xt[:].rearrange("p (t b) -> p t b", t=2),
                    in_=x_d.rearrange("(t p) b -> p t b", t=2)[:, :, s : s + w],
                )
                xt_tiles.append(xt)

            # Get the x stream going before anything else.
            for c in range(len(CHUNKS)):
                load(c)

            # PE warm-up: ~6us of junk matmuls so the HAM clock gate opens
            # (1.2 -> 2.4 GHz) before the first real matmul arrives, with no
            # >3.4us PE-idle window in between.
            warm = const_pool.tile([128, 128], bf16)
            nc.gpsimd.memset(warm[:], 1.0)
            warm_ps = po_pool.tile([128, 1024], f32, tag="po", name="warm_ps")
            for _ in range(90):
                nc.tensor.matmul(
                    warm_ps[:, 0:64], warm[:], warm[:, 0:64], start=True, stop=True
                )

            # DoubleRow stationary tiles, one per o-half s, laid out
            # [k', (t, oo)] with t = k-half:
            #   ktd0 = [ 2*sign(wrT) | -2*sign(wiT) ]   (o in [0,128))
            #   ktd1 = [ 2*sign(wiT) |  2*sign(wrT) ]   (o in [128,256))
            # sign(w + eps) = (w >= -eps)*2 - 1, two tensor_scalar passes on
            # GPSIMD (keeps the DVE free for the first binarize).
            w_sb = const_pool.tile([128, 256], f32)
            nc.scalar.dma_start(out=w_sb[:, 0:128], in_=wrt_d[:])
            nc.scalar.dma_start(out=w_sb[:, 128:256], in_=wit_d[:])
            kt_t = const_pool.tile([128, 512], bf16)
            # order: [wrT+, wiT-, wiT+, wrT+] scaled {0,+-4}
            nc.vector.tensor_scalar(kt_t[:, 0:128], w_sb[:, 0:128], -EPS, 4.0, Alu.is_ge, Alu.mult)
            nc.vector.tensor_scalar(kt_t[:, 128:256], w_sb[:, 128:256], -EPS, -4.0, Alu.is_ge, Alu.mult)
            nc.vector.tensor_scalar(kt_t[:, 256:384], w_sb[:, 128:256], -EPS, 4.0, Alu.is_ge, Alu.mult)
            nc.vector.tensor_scalar(kt_t[:, 384:512], w_sb[:, 0:128], -EPS, 4.0, Alu.is_ge, Alu.mult)
            # Stationary layout [p, (s, t, o)]: for each o-half s the
            # weights AP is [Ki, (t: n=2, step 128), (o: step 1)] -- the
            # DoubleRow format (first free dim = the 2 k-halves).
            ktd = const_pool.tile([128, 512], fp8)
            nc.vector.tensor_scalar(ktd[:, 0:128], kt_t[:, 0:128], -2.0, None, Alu.add)
            nc.vector.tensor_scalar(ktd[:, 128:256], kt_t[:, 128:256], 2.0, None, Alu.add)
            nc.vector.tensor_scalar(ktd[:, 256:512], kt_t[:, 256:512], -2.0, None, Alu.add)
            ktdv = ktd[:].rearrange("p (s t o) -> p s t o", s=2, t=2)

            for c in range(len(CHUNKS)):
                s, w = starts[c], CHUNKS[c]
                xbt = xb_pool.tile([128, 2 * w], fp8, tag="xb")
                xbv = xbt[:].rearrange("p (t b) -> p t b", t=2)
                # One-shot binarize: (x >= 0) - 0.5 -> {-0.5, +0.5} fp8.
                nc.vector.tensor_scalar(
                    xbt[:], xt_tiles[c][:], 0.0, 0.5, Alu.is_ge, Alu.subtract
                )
                ot = o_pool.tile([128, 2 * w], i8, tag="ot")
                otv = ot[:].rearrange("p (os b) -> p os b", os=2)
                # po tiles of up to 1024 cols (2 PSUM banks)
                widths = []
                b = 0
                while b < w:
                    wd = min(1024, w - b)
                    widths.append((b, wd))
                    b += wd
                for os_half in range(2):
                    pos = [
                        po_pool.tile([128, wd], f32, tag="po", name=f"po_{c}_{os_half}_{j}")
                        for j, (_, wd) in enumerate(widths)
                    ]
                    for j, (jb, wd) in enumerate(widths):
                        for q in range(wd // 512):
                            # DoubleRow: rhs = [p, (t: 2, step w), (b: 512)],
                            # stationary holds both k-halves; one matmul =
                            # full k=256 contraction for 512 batch rows.
                            b0 = jb + q * 512
                            nc.tensor.matmul(
                                pos[j][:, q * 512 : (q + 1) * 512],
                                ktdv[:, os_half, :, :],
                                xbv[:, :, b0 : b0 + 512],
                                start=True,
                                stop=True,
                                perf_mode=DR,
                            )
                    for j, (jb, wd) in enumerate(widths):
                        dst = otv[:, os_half, jb : jb + wd]
                        # ~6 of 32 copies on DVE (emitted before the next
                        # chunk's binarize in the FIFO), rest on ACT.
                        on_dve = os_half == 0 and j == 0 and w == 2048
                        if on_dve:
                            nc.vector.tensor_scalar(dst, pos[j][:], 0.5, None, Alu.mult)
                        else:
                            nc.scalar.activation(dst, pos[j][:], Copy, bias=0.0, scale=0.5)
                nc.gpsimd.dma_start(
                    out=out_d.rearrange("(os p) b -> p os b", os=2)[:, :, s : s + w],
                    in_=otv,
                )

    nc.compile()
    return nc


def get_nc(rows_per_core=ROWS_PER_CORE):
    if rows_per_core not in _NC_CACHE:
        _NC_CACHE[rows_per_core] = _build_nc(rows_per_core)
    return _NC_CACHE[rows_per_core]


def kernel(x, weight_real, weight_imag, trace=False, tmpdir=None):
    from concourse import bass_utils

    x = np.asarray(x, dtype=np.float32)
    wr = np.asarray(weight_real, dtype=np.float32)
    wi = np.asarray(weight_imag, dtype=np.float32)
    assert x.shape == (B_TOTAL, K2) and wr.shape == (FAN, FAN) and wi.shape == (FAN, FAN)

    # Fold the +eps into the bf16 cast: sign(bf16(x + eps)) == sign(x + eps)
    # (round-to-nearest never crosses 0; exact-0 results go +1 via the
    # device-side >= 0 test, matching sign(0 + eps)).  Feed each core its
    # shard k-major ([256, 16384]); weights go in pre-transposed.
    x_bf = (x + np.float32(EPS)).astype(ml_dtypes.bfloat16)
    xp = np.ascontiguousarray(
        x_bf.reshape(N_CORES, ROWS_PER_CORE, K2).transpose(0, 2, 1)
    )
    wrt = np.ascontiguousarray(wr.T)
    wit = np.ascontiguousarray(wi.T)

    nc = get_nc()
    in_maps = [
        {"x": xp[i], "weight_real_t": wrt, "weight_imag_t": wit}
        for i in range(N_CORES)
    ]
    res = bass_utils.run_bass_kernel_spmd(
        nc, in_maps, core_ids=list(range(N_CORES)), trace=trace, tmpdir=tmpdir
    )
    # out_d[o, b] = out[b, o]/2 per core: untranspose and upcast.
    out = np.empty((B_TOTAL, K2), dtype=np.float32)
    for i in range(N_CORES):
        np.multiply(
            res.results[i]["out"].T, np.float32(2.0),
            out=out[i * ROWS_PER_CORE : (i + 1) * ROWS_PER_CORE],
        )
    if trace:
        return out, res
    return out
